# revision 1
# baseline (speedup 1.0000x reference)
"""Trainium2 Bass kernel for nn_EquivarianceNetwork (grouped 4-layer MLP).

Math (per sample b, TWO_N=16 groups, D=64):
  xr = x.reshape(B, 16, 64)
  scalars[b, n, m] = <xr[b,n], xr[b,m]>                  # [B, 256]
  per group l: h = tanh(...W0/W1/W2...), coeffs = h @ W3 + b3   # [B, 16]
  out[b, l*64:(l+1)*64] = sum_n coeffs[l,b,n] * xr[b,n]

Distribution: data-parallel over batch across 8 cores (weights replicated).
Per core B_local = 2048.

Engine plan per core:
  - PE: all GEMMs in float32r (fp32 data, ~TF32 matmul precision, 1 cyc/row
    at N=512) with feature-major activations; small transposes.
  - ACT: tanh+bias (PSUM->SBUF, float32r out), L3 bias add, PSUM->SBUF copies.
  - DVE: Gram reduces + 2 mult-deltas + mirrors, final-stage reduces and
    half the final mults.
  - GPSIMD: remaining Gram mult-deltas, other half of final mults.
  - Weights streamed from HBM per group l, double buffered; biases preloaded.
"""
import numpy as np

import concourse.bass as bass
import concourse.mybir as mybir
import concourse.tile as tile
from concourse import bacc
from concourse.bass_utils import run_bass_kernel_spmd
from concourse.masks import make_identity

F32 = mybir.dt.float32
F16 = mybir.dt.float16
F32R = mybir.dt.float32r
TANH = mybir.ActivationFunctionType.Tanh

N_CORES = 8
B = 16384
TWO_N = 16
D = 64
B_LOC = B // N_CORES          # 2048
N_SUB = B_LOC // 128          # 16 subtiles of 128 samples
N_BT = B_LOC // 512           # 4 batch tiles of 512 (matmul free dim)
H = 1024                      # hidden width
K_IN = 256                    # 16*16 scalars


def _build_program():
    nc = bacc.Bacc()

    x = nc.declare_dram_parameter("x", [B_LOC, TWO_N * D], F32, isOutput=False)
    W0 = nc.declare_dram_parameter("W0", [TWO_N, K_IN, H], F32R, isOutput=False)
    W1 = nc.declare_dram_parameter("W1", [TWO_N, H, H], F32R, isOutput=False)
    W2 = nc.declare_dram_parameter("W2", [TWO_N, H, H], F32R, isOutput=False)
    W3 = nc.declare_dram_parameter("W3", [TWO_N, H, TWO_N], F32R, isOutput=False)
    b0 = nc.declare_dram_parameter("b0", [TWO_N, H], F32, isOutput=False)
    b1 = nc.declare_dram_parameter("b1", [TWO_N, H], F32, isOutput=False)
    b2 = nc.declare_dram_parameter("b2", [TWO_N, H], F32, isOutput=False)
    b3 = nc.declare_dram_parameter("b3", [TWO_N, TWO_N], F32, isOutput=False)
    y = nc.declare_dram_parameter("y", [B_LOC, TWO_N * D], F32, isOutput=True)

    with tile.TileContext(nc) as tc:
        with tc.tile_pool(name="res", bufs=1) as res, \
             tc.tile_pool(name="xg", bufs=5) as xgp, \
             tc.tile_pool(name="work", bufs=2) as wk, \
             tc.tile_pool(name="w0p", bufs=2) as w0p, \
             tc.tile_pool(name="w12p", bufs=4) as w12p, \
             tc.tile_pool(name="w3p", bufs=2) as w3p, \
             tc.tile_pool(name="hp", bufs=2) as hp, \
             tc.tile_pool(name="fin", bufs=4) as finp, \
             tc.tile_pool(name="ps", bufs=4, space="PSUM") as ps:

            ident = res.tile([128, 128], F32)
            make_identity(nc, ident)

            # ---- biases: preload all groups once, transposed on PE ----
            # b012_all[p, li, ot, l] = b_li[l, ot*128 + p]
            b012_all = res.tile([128, 3, 8, TWO_N], F32)
            b3_all = res.tile([16, TWO_N], F32)   # [n, l]
            for li, bsrc in enumerate((b0, b1, b2)):
                bnat = wk.tile([TWO_N, H], F32, name=f"bnat{li}", tag="bnat")
                nc.sync.dma_start(out=bnat, in_=bsrc[:, :])
                for ot in range(8):
                    pt = ps.tile([128, 128], F32, name="tpb", tag="tp", bufs=2)
                    nc.tensor.transpose(
                        pt[:, 0:TWO_N], bnat[:, 128 * ot:128 * (ot + 1)],
                        ident[0:TWO_N, 0:TWO_N])
                    nc.scalar.copy(b012_all[:, li, ot, :], pt[:, 0:TWO_N])
            b3nat = wk.tile([TWO_N, TWO_N], F32, name="b3nat", tag="bnat")
            nc.sync.dma_start(out=b3nat, in_=b3[:, :])
            pt = ps.tile([128, 128], F32, name="tpb3", tag="tp", bufs=2)
            nc.tensor.transpose(pt[0:TWO_N, 0:TWO_N], b3nat[:, :],
                                ident[0:TWO_N, 0:TWO_N])
            nc.scalar.copy(b3_all[:, :], pt[0:TWO_N, 0:TWO_N])

            # resident: transposed scalars [256, B_LOC] as 2 partition tiles
            scalT = [res.tile([128, B_LOC], F32R, name=f"scalT{i}")
                     for i in range(2)]
            # resident: coeffs batch-major per subtile [128, 256] (col l*16+n)
            coeff = [res.tile([128, 256], F32, name=f"coeff{s}")
                     for s in range(N_SUB)]

            # ---------------- Gram for one subtile of 128 samples ----------
            def gram(s):
                xg = xgp.tile([128, TWO_N * D], F32, name="xg", tag="xg")
                nc.sync.dma_start(out=xg, in_=x[128 * s:128 * (s + 1), :])
                # fp16 copy of x: 16-bit ops run the DVE/GP 2x perf mode for
                # the O(B*256*64) Gram products; scalars stay fp32 accurate
                # to ~5e-4 which is far below the fp32r matmul noise.
                xh = wk.tile([128, TWO_N * D], F16, name="xh", tag="xh")
                nc.scalar.copy(xh, xg)
                sbm = wk.tile([128, K_IN], F32, name="sbm", tag="sbm")
                prod = wk.tile([128, TWO_N * D], F16, name="prod", tag="prod")
                if s < 2:
                    # first use of each sbm slot: zero the mirror columns so
                    # the (m>n) garbage cols are finite (W0 is host-folded
                    # into the upper triangle; lower-triangle weights are 0).
                    nc.gpsimd.memset(sbm[:, :], 0.0)
                for dl in range(TWO_N):
                    npair = TWO_N - dl
                    meng = nc.vector if dl < 4 else nc.gpsimd
                    meng.tensor_mul(
                        prod[:, 0:npair * D],
                        xh[:, 0:npair * D],
                        xh[:, dl * D:(dl + npair) * D],
                    )
                    dst = bass.AP(tensor=sbm.tensor, offset=sbm.offset + dl,
                                  ap=[sbm.ap[0], [17, npair]])
                    nc.vector.tensor_reduce(
                        dst, prod[:, 0:npair * D].rearrange(
                            "p (n d) -> p n d", d=D),
                        axis=mybir.AxisListType.X, op=mybir.AluOpType.add)
                for i in range(2):
                    pt = ps.tile([128, 128], F32, name="tp", tag="tp", bufs=2)
                    nc.tensor.transpose(
                        pt[:, :], sbm[:, 128 * i:128 * (i + 1)], ident)
                    nc.scalar.copy(
                        scalT[i][:, 128 * s:128 * (s + 1)], pt[:, :])

            # ---- final contraction for one (l, subtile):
            # y[bsub, l*64+d] = sum_n coeff[b, 16l+n] * x[b, 64n+d]
            def final_unit(l, s):
                xg = xgp.tile([128, TWO_N * D], F32, name="xg2", tag="xg")
                nc.sync.dma_start(out=xg, in_=x[128 * s:128 * (s + 1), :])
                prod = wk.tile([128, TWO_N * D], F32, name="prod2", tag="prod")
                c = coeff[s]
                in1 = bass.AP(tensor=c.tensor, offset=c.offset + 16 * l,
                              ap=[c.ap[0], [1, TWO_N], [0, D]])
                meng = (nc.vector if s % 2 == 0 else nc.gpsimd) \
                    if l == TWO_N - 1 else \
                    (nc.vector if s % 4 != 3 else nc.gpsimd)
                meng.tensor_mul(
                    prod[:, :].rearrange("p (n d) -> p n d", d=D),
                    xg[:, :].rearrange("p (n d) -> p n d", d=D),
                    in1)
                meng.tensor_add(prod[:, 0:512], prod[:, 0:512], prod[:, 512:1024])
                meng.tensor_add(prod[:, 0:256], prod[:, 0:256], prod[:, 256:512])
                meng.tensor_add(prod[:, 0:128], prod[:, 0:128], prod[:, 128:256])
                fcol = finp.tile([128, D], F32, name="fcol", tag="fcol")
                meng.tensor_add(fcol[:, :], prod[:, 0:D], prod[:, D:2 * D])
                nc.sync.dma_start(
                    out=y[128 * s:128 * (s + 1), D * l:D * (l + 1)],
                    in_=fcol[:, :])

            # ---------------- Phase B: grouped MLP ----------------
            # The first Gram group is hoisted ahead of the l=0 weight
            # stream; inside l==0, group k+1 is emitted after MLP bt k so
            # the PE stream never waits on a group it doesn't need yet.
            for s in range(4):
                gram(s)

            for l in range(TWO_N):
                w0t = w0p.tile([128, 2, H], F32R, name="w0t", tag="w0")
                nc.sync.dma_start(
                    out=w0t,
                    in_=W0[l, :, :].rearrange("(t p) m -> p t m", p=128))
                w1h = []
                w2h = []
                for hname, Wsrc, lst in (("w1", W1, w1h), ("w2", W2, w2h)):
                    for half in range(2):
                        wt = w12p.tile([128, 4, H], F32R,
                                       name=f"{hname}{half}", tag="w12")
                        nc.sync.dma_start(
                            out=wt,
                            in_=Wsrc[l, 512 * half:512 * (half + 1), :]
                            .rearrange("(t p) m -> p t m", p=128))
                        lst.append(wt)
                w3t = w3p.tile([128, 8, TWO_N], F32R, name="w3t", tag="w3")
                nc.sync.dma_start(
                    out=w3t,
                    in_=W3[l, :, :].rearrange("(t p) m -> p t m", p=128))

                for bt in range(N_BT):
                    bs = 512 * bt
                    # L0: scalT -> h0
                    h0 = hp.tile([128, 8, 512], F32R, name="h0", tag="h")
                    for ot in range(8):
                        pt = ps.tile([128, 512], F32, name="mlp", tag="mlp",
                                     bufs=5)
                        for kt in range(2):
                            nc.tensor.matmul(
                                pt[:, :],
                                w0t[:, kt, 128 * ot:128 * (ot + 1)],
                                scalT[kt][:, bs:bs + 512],
                                start=(kt == 0), stop=(kt == 1))
                        nc.scalar.activation(
                            h0[:, ot, :], pt[:, :], TANH,
                            bias=b012_all[:, 0, ot, l:l + 1])
                    # L1, L2
                    hin = h0
                    for li, whalves in ((1, w1h), (2, w2h)):
                        hout = hp.tile([128, 8, 512], F32R,
                                       name=f"h{li}", tag="h")
                        for ot in range(8):
                            pt = ps.tile([128, 512], F32, name="mlp",
                                         tag="mlp", bufs=5)
                            for kt in range(8):
                                nc.tensor.matmul(
                                    pt[:, :],
                                    whalves[kt // 4][:, kt % 4,
                                                     128 * ot:128 * (ot + 1)],
                                    hin[:, kt, :],
                                    start=(kt == 0), stop=(kt == 7))
                            nc.scalar.activation(
                                hout[:, ot, :], pt[:, :], TANH,
                                bias=b012_all[:, li, ot, l:l + 1])
                        hin = hout
                    # L3 -> coeffs [16, 512] + bias, transpose to batch-major
                    p3 = ps.tile([16, 512], F32, name="p3", tag="p3", bufs=1)
                    for kt in range(8):
                        nc.tensor.matmul(p3[:, :], w3t[:, kt, :],
                                         hin[:, kt, :],
                                         start=(kt == 0), stop=(kt == 7))
                    csb = wk.tile([16, 512], F32, name="csb", tag="csb")
                    nc.scalar.add(csb[:, :], p3[:, :], b3_all[:, l:l + 1])
                    for j in range(4):
                        tp = ps.tile([128, 16], F32, name="tp2", tag="tp",
                                     bufs=2)
                        nc.tensor.transpose(
                            tp[:, 0:16], csb[:, 128 * j:128 * (j + 1)],
                            ident[0:16, 0:16])
                        sub = 4 * bt + j
                        nc.scalar.copy(
                            coeff[sub][:, 16 * l:16 * (l + 1)], tp[:, 0:16])

                    if l == 0:
                        # l=0 is Gram-bound: emit the next Gram group here
                        # and defer finals to the end of the group loop.
                        if bt < 3:
                            for s in range(4 * bt + 4, 4 * bt + 8):
                                gram(s)
                    else:
                        # finals for this bt's subtiles (their coeff cols
                        # are ready); spreads DVE work and the xg DMAs
                        for s in range(4 * bt, 4 * bt + 4):
                            final_unit(l, s)

                if l == 0:
                    for s in range(N_SUB):
                        final_unit(l, s)

    nc.finalize()
    return nc


_NC = None


def build_in_maps(x, W0, b0, W1, b1, W2, b2, W3, b3):
    x = np.ascontiguousarray(np.asarray(x, dtype=np.float32))
    # Fold W0 over the symmetric scalar pairs: scalars[b,(n,m)] == [b,(m,n)],
    # and the kernel only materializes the upper triangle (col 16n+m, n<=m).
    # h0 = scal @ W0 is preserved exactly by moving the lower-triangle
    # weights onto their mirrored counterpart and zeroing them.
    W0f = np.asarray(W0, np.float32).reshape(TWO_N, TWO_N, TWO_N, H).copy()
    for n in range(TWO_N):
        for m in range(n + 1, TWO_N):
            W0f[:, n, m, :] += W0f[:, m, n, :]
            W0f[:, m, n, :] = 0.0
    W0f = W0f.reshape(TWO_N, K_IN, H)
    shared = {
        "W0": np.ascontiguousarray(W0f),
        "W1": np.ascontiguousarray(np.asarray(W1, np.float32)),
        "W2": np.ascontiguousarray(np.asarray(W2, np.float32)),
        "W3": np.ascontiguousarray(np.asarray(W3, np.float32)),
        "b0": np.ascontiguousarray(np.asarray(b0, np.float32)),
        "b1": np.ascontiguousarray(np.asarray(b1, np.float32)),
        "b2": np.ascontiguousarray(np.asarray(b2, np.float32)),
        "b3": np.ascontiguousarray(np.asarray(b3, np.float32)),
    }
    in_maps = []
    for c in range(N_CORES):
        m = dict(shared)
        m["x"] = x[B_LOC * c:B_LOC * (c + 1), :]
        in_maps.append(m)
    return in_maps


def kernel(x, W0, b0, W1, b1, W2, b2, W3, b3):
    global _NC
    if _NC is None:
        _NC = _build_program()
    in_maps = build_in_maps(x, W0, b0, W1, b1, W2, b2, W3, b3)
    res = run_bass_kernel_spmd(_NC, in_maps, list(range(N_CORES)))
    return np.concatenate([res.results[c]["y"] for c in range(N_CORES)],
                          axis=0)



# revision 4
# speedup vs baseline: 1.0268x; 1.0268x over previous
"""Trainium2 Bass kernel for nn_EquivarianceNetwork (grouped 4-layer MLP).

Math (per sample b, TWO_N=16 groups, D=64):
  xr = x.reshape(B, 16, 64)
  scalars[b, n, m] = <xr[b,n], xr[b,m]>                  # symmetric, 136 distinct
  per group l: h = tanh(...W0/W1/W2...), coeffs = h @ W3 + b3   # [B, 16]
  out[b, l*64:(l+1)*64] = sum_n coeffs[l,b,n] * xr[b,n]

Distribution: data-parallel over batch across 8 cores (weights replicated).
Per core B_local = 2048.

v2 design (PE floor ~2.1ms at ~1 row/cycle; fp8/DoubleRow measured to give
no real-HW MAC-rate gain, so everything is fp16):
  - All MLP matmuls in fp16 (measured ~5% faster/row than f32r, half the
    weight DMA + SBUF); accumulation stays fp32 in PSUM.
  - Gram scalars triangle-packed to 136 rows (128+8 k-tiles); products and
    segmented reduces in fp16 (DVE 2x mode), mults split DVE/GPSIMD by
    pair-width; reduces are DVE-only (GPSIMD can't do X-axis reduces).
  - x resident in SBUF as fp16 (xhall) -> no per-(l,s) re-DMA for finals.
  - finals: fp16 multiply + fp16 2x-mode tree adds, split DVE/GPSIMD.
  - L3 coeff transposes + copies + finals deferred into the next (l,bt)
    emission slot so PE never ping-pongs with ACT within a group.
  - All 16 gram subtiles emitted up-front on DVE/GP; l=0 consumes them
    batch-by-batch with PE transposes emitted just-in-time.
"""
import numpy as np
from contextlib import ExitStack
import ml_dtypes

import concourse.bass as bass
import concourse.mybir as mybir
import concourse.tile as tile
from concourse import bacc
from concourse.bass_utils import run_bass_kernel_spmd
from concourse.masks import make_identity

F32 = mybir.dt.float32
F16 = mybir.dt.float16
TANH = mybir.ActivationFunctionType.Tanh

N_CORES = 8
B = 16384
TWO_N = 16
D = 64
B_LOC = B // N_CORES          # 2048
N_SUB = B_LOC // 128          # 16 subtiles of 128 samples
N_BT = B_LOC // 512           # 4 batch tiles of 512 (matmul free dim)
H = 1024                      # hidden width
K_TRI = 136                   # packed upper-triangle scalar count
CUM = [dl * TWO_N - dl * (dl - 1) // 2 for dl in range(TWO_N + 1)]
SPLIT_DL = 6                  # gram mults: dl < SPLIT_DL on DVE, rest GPSIMD


def _build_program():
    nc = bacc.Bacc()

    x = nc.declare_dram_parameter("x", [B_LOC, TWO_N * D], F32, isOutput=False)
    W0a = nc.declare_dram_parameter("W0a", [TWO_N, 128, H], F16, isOutput=False)
    W0b = nc.declare_dram_parameter("W0b", [TWO_N, 8, H], F16, isOutput=False)
    W1 = nc.declare_dram_parameter("W1", [TWO_N, H, H], F16, isOutput=False)
    W2 = nc.declare_dram_parameter("W2", [TWO_N, H, H], F16, isOutput=False)
    W3 = nc.declare_dram_parameter("W3", [TWO_N, H, TWO_N], F16, isOutput=False)
    b0 = nc.declare_dram_parameter("b0", [TWO_N, H], F32, isOutput=False)
    b1 = nc.declare_dram_parameter("b1", [TWO_N, H], F32, isOutput=False)
    b2 = nc.declare_dram_parameter("b2", [TWO_N, H], F32, isOutput=False)
    b3 = nc.declare_dram_parameter("b3", [TWO_N, TWO_N], F32, isOutput=False)
    y = nc.declare_dram_parameter("y", [B_LOC, TWO_N * D], F32, isOutput=True)

    with tile.TileContext(nc) as tc, ExitStack() as ctx:
        pool = lambda *a, **kw: ctx.enter_context(tc.tile_pool(*a, **kw))
        res = pool(name="res", bufs=1)
        xgp = pool(name="xg", bufs=4)
        wk = pool(name="wk", bufs=2)
        sbmp = pool(name="sbm", bufs=16)
        w0ap = pool(name="w0a", bufs=2)
        w0bp = pool(name="w0b", bufs=2)
        w12p = pool(name="w12", bufs=8)
        w3p = pool(name="w3p", bufs=2)
        hp = pool(name="hp", bufs=3)
        pvp = pool(name="pv", bufs=3)
        pgp = pool(name="pg", bufs=3)
        finp = pool(name="fin", bufs=4)
        csbp = pool(name="csb", bufs=2)
        ps = pool(name="ps", bufs=4, space="PSUM")
        tps = pool(name="tp", bufs=2, space="PSUM")
        p3s = pool(name="p3", bufs=2, space="PSUM")
        if True:

            ident = res.tile([128, 128], F32)
            make_identity(nc, ident)
            identh = res.tile([128, 128], F16)
            make_identity(nc, identh)

            # ---- biases: preload all groups once, transposed on PE ----
            # b012_all[p, li, ot, l] = b_li[l, ot*128 + p]
            b012_all = res.tile([128, 3, 8, TWO_N], F32)
            b3_all = res.tile([16, TWO_N], F32)   # [n, l]
            for li, bsrc in enumerate((b0, b1, b2)):
                bnat = wk.tile([TWO_N, H], F32, name=f"bnat{li}", tag="bnat")
                nc.sync.dma_start(out=bnat, in_=bsrc[:, :])
                for ot in range(8):
                    pt = tps.tile([128, 128], F32, name="tpb", tag="tp")
                    nc.tensor.transpose(
                        pt[:, 0:TWO_N], bnat[:, 128 * ot:128 * (ot + 1)],
                        ident[0:TWO_N, 0:TWO_N])
                    nc.scalar.copy(b012_all[:, li, ot, :], pt[:, 0:TWO_N])
            b3nat = wk.tile([TWO_N, TWO_N], F32, name="b3nat", tag="bnat")
            nc.sync.dma_start(out=b3nat, in_=b3[:, :])
            ptb = tps.tile([128, 128], F32, name="tpb3", tag="tp")
            nc.tensor.transpose(ptb[0:TWO_N, 0:TWO_N], b3nat[:, :],
                                ident[0:TWO_N, 0:TWO_N])
            nc.scalar.copy(b3_all[:, :], ptb[0:TWO_N, 0:TWO_N])

            # resident fp16 x (used by gram mults and finals)
            xhall = res.tile([128, N_SUB, TWO_N * D], F16)
            # resident transposed scalars [136, B_LOC] fp16 (128 + 8 rows)
            scalTa = res.tile([128, B_LOC], F16, name="scalTa")
            scalTb = res.tile([8, B_LOC], F16, name="scalTb")
            # resident coeffs batch-major per subtile [128, 256] fp16
            coeff = [res.tile([128, 256], F16, name=f"coeff{s}")
                     for s in range(N_SUB)]
            sbm_tiles = [None] * N_SUB

            # ---------------- Gram DVE/GP part for one subtile ----------
            def gram_dve(s):
                xg = xgp.tile([128, TWO_N * D], F32, name="xg", tag="xg")
                nc.sync.dma_start(out=xg, in_=x[128 * s:128 * (s + 1), :])
                xh = xhall[:, s, :]
                nc.scalar.copy(xh, xg)
                sbm = sbmp.tile([128, K_TRI], F16, name=f"sbm{s}", tag="sbm")
                sbm_tiles[s] = sbm
                for dl in range(TWO_N):
                    npair = TWO_N - dl
                    meng = nc.vector if dl < SPLIT_DL else nc.gpsimd
                    pool = pvp if dl < SPLIT_DL else pgp
                    prod = pool.tile([128, TWO_N * D], F16, name="prod",
                                     tag="prod")
                    meng.tensor_mul(
                        prod[:, 0:npair * D],
                        xh[0:128, 0:npair * D],
                        xh[0:128, dl * D:(dl + npair) * D],
                    )
                    with nc.allow_low_precision("fp16 gram accum, ~2e-3 ok"):
                        nc.vector.tensor_reduce(
                            sbm[:, CUM[dl]:CUM[dl] + npair],
                            prod[:, 0:npair * D].rearrange(
                                "p (n d) -> p n d", d=D),
                            axis=mybir.AxisListType.X, op=mybir.AluOpType.add)

            # ---- Gram PE part: transpose sbm -> scalTa/scalTb columns ----
            def gram_pe(s):
                sbm = sbm_tiles[s]
                pt = tps.tile([128, 128], F16, name="tpg", tag="tp")
                nc.tensor.transpose(pt[:, :], sbm[:, 0:128], identh)
                nc.scalar.copy(scalTa[:, 128 * s:128 * (s + 1)], pt[:, :])
                ptb2 = tps.tile([8, 128], F16, name="tpg8", tag="tp")
                nc.tensor.transpose(ptb2[:, :], sbm[:, 128:K_TRI],
                                    identh)
                nc.scalar.copy(scalTb[:, 128 * s:128 * (s + 1)], ptb2[:, :])

            # ---- final contraction for one (l, subtile):
            # y[bsub, l*64+d] = sum_n coeff[b, 16l+n] * xh[b, 64n+d]
            def final_unit(l, s):
                c = coeff[s]
                in1 = bass.AP(tensor=c.tensor, offset=c.offset + 16 * l,
                              ap=[c.ap[0], [1, TWO_N], [0, D]])
                meng = nc.gpsimd if s % 4 == 3 else nc.vector
                pool = pgp if s % 4 == 3 else pvp
                prod = pool.tile([128, TWO_N * D], F16, name="prod2",
                                 tag="prod")
                meng.tensor_mul(
                    prod[:, :].rearrange("p (n d) -> p n d", d=D),
                    xhall[:, s, :].rearrange("p (n d) -> p n d", d=D),
                    in1)
                meng.tensor_add(prod[:, 0:512], prod[:, 0:512],
                                prod[:, 512:1024])
                meng.tensor_add(prod[:, 0:256], prod[:, 0:256],
                                prod[:, 256:512])
                meng.tensor_add(prod[:, 0:128], prod[:, 0:128],
                                prod[:, 128:256])
                fcol = finp.tile([128, D], F32, name="fcol", tag="fcol")
                meng.tensor_add(fcol[:, :], prod[:, 0:D], prod[:, D:2 * D])
                nc.sync.dma_start(
                    out=y[128 * s:128 * (s + 1), D * l:D * (l + 1)],
                    in_=fcol[:, :])

            # ---------------- emit all gram compute up front ------------
            for s in range(N_SUB):
                gram_dve(s)

            # deferred PE/ACT/DVE work from the previous (l, bt)
            pending = []

            def flush_pending():
                nonlocal pending
                for fn in pending:
                    fn()
                pending = []

            for l in range(TWO_N):
                w0at = w0ap.tile([128, H], F16, name="w0at", tag="w0a")
                nc.sync.dma_start(out=w0at, in_=W0a[l, :, :])
                w0bt = w0bp.tile([8, H], F16, name="w0bt", tag="w0b")
                nc.sync.dma_start(out=w0bt, in_=W0b[l, :, :])
                w1h = []
                w2h = []
                for hname, Wsrc, lst in (("w1", W1, w1h), ("w2", W2, w2h)):
                    for half in range(2):
                        wt = w12p.tile([128, 4, H], F16,
                                       name=f"{hname}{half}", tag="w12")
                        nc.sync.dma_start(
                            out=wt,
                            in_=Wsrc[l, 512 * half:512 * (half + 1), :]
                            .rearrange("(t p) m -> p t m", p=128))
                        lst.append(wt)
                w3t = w3p.tile([128, 8, TWO_N], F16, name="w3t", tag="w3")
                nc.sync.dma_start(
                    out=w3t,
                    in_=W3[l, :, :].rearrange("(t p) m -> p t m", p=128))

                for bt in range(N_BT):
                    bs = 512 * bt
                    flush_pending()
                    if l == 0:
                        for s in range(4 * bt, 4 * bt + 4):
                            gram_pe(s)
                    # L0: scalT (136 = 128 + 8 rows) -> h0
                    h0 = hp.tile([128, 8, 512], F16, name="h0", tag="h")
                    for ot in range(8):
                        pt = ps.tile([128, 512], F32, name="mlp", tag="mlp")
                        nc.tensor.matmul(
                            pt[:, :], w0at[:, 128 * ot:128 * (ot + 1)],
                            scalTa[:, bs:bs + 512], start=True, stop=False)
                        nc.tensor.matmul(
                            pt[:, :], w0bt[:, 128 * ot:128 * (ot + 1)],
                            scalTb[:, bs:bs + 512], start=False, stop=True)
                        nc.scalar.activation(
                            h0[:, ot, :], pt[:, :], TANH,
                            bias=b012_all[:, 0, ot, l:l + 1])
                    # L1, L2
                    hin = h0
                    for li, whalves in ((1, w1h), (2, w2h)):
                        hout = hp.tile([128, 8, 512], F16,
                                       name=f"h{li}", tag="h")
                        for ot in range(8):
                            pt = ps.tile([128, 512], F32, name="mlp",
                                         tag="mlp")
                            for kt in range(8):
                                nc.tensor.matmul(
                                    pt[:, :],
                                    whalves[kt // 4][:, kt % 4,
                                                     128 * ot:128 * (ot + 1)],
                                    hin[:, kt, :],
                                    start=(kt == 0), stop=(kt == 7))
                            nc.scalar.activation(
                                hout[:, ot, :], pt[:, :], TANH,
                                bias=b012_all[:, li, ot, l:l + 1])
                        hin = hout
                    # L3 -> coeffs [16, 512] + bias
                    p3 = p3s.tile([16, 512], F32, name="p3", tag="p3")
                    for kt in range(8):
                        nc.tensor.matmul(p3[:, :], w3t[:, kt, :],
                                         hin[:, kt, :],
                                         start=(kt == 0), stop=(kt == 7))
                    csb = csbp.tile([16, 512], F32, name="csb", tag="csb")
                    nc.scalar.add(csb[:, :], p3[:, :], b3_all[:, l:l + 1])

                    # transpose to batch-major + finals: deferred into the
                    # next (l, bt) slot so PE/ACT don't ping-pong here.
                    def make_tail(l=l, bt=bt, csb=csb):
                        def tail():
                            for j in range(4):
                                tp = tps.tile([128, 16], F32, name="tp2",
                                              tag="tp")
                                nc.tensor.transpose(
                                    tp[:, 0:16],
                                    csb[:, 128 * j:128 * (j + 1)],
                                    ident[0:16, 0:16])
                                sub = 4 * bt + j
                                nc.scalar.copy(
                                    coeff[sub][:, 16 * l:16 * (l + 1)],
                                    tp[:, 0:16])
                            for s in range(4 * bt, 4 * bt + 4):
                                final_unit(l, s)
                        return tail
                    pending.append(make_tail())

            flush_pending()

    nc.finalize()
    return nc


_NC = None


def build_in_maps(x, W0, b0, W1, b1, W2, b2, W3, b3):
    x = np.ascontiguousarray(np.asarray(x, dtype=np.float32))
    # Fold W0 over the symmetric scalar pairs and pack the upper triangle
    # in (dl, n) band order to match the gram layout:
    # tri row CUM[dl]+n  <->  scalar pair (n, n+dl).
    W0f = np.asarray(W0, np.float32).reshape(TWO_N, TWO_N, TWO_N, H)
    W0t = np.zeros((TWO_N, K_TRI, H), np.float32)
    for dl in range(TWO_N):
        for n in range(TWO_N - dl):
            if dl == 0:
                W0t[:, CUM[0] + n] = W0f[:, n, n]
            else:
                W0t[:, CUM[dl] + n] = (W0f[:, n, n + dl]
                                       + W0f[:, n + dl, n])
    f16 = np.float16
    shared = {
        "W0a": np.ascontiguousarray(W0t[:, 0:128].astype(f16)),
        "W0b": np.ascontiguousarray(W0t[:, 128:K_TRI].astype(f16)),
        "W1": np.ascontiguousarray(np.asarray(W1, np.float32).astype(f16)),
        "W2": np.ascontiguousarray(np.asarray(W2, np.float32).astype(f16)),
        "W3": np.ascontiguousarray(np.asarray(W3, np.float32).astype(f16)),
        "b0": np.ascontiguousarray(np.asarray(b0, np.float32)),
        "b1": np.ascontiguousarray(np.asarray(b1, np.float32)),
        "b2": np.ascontiguousarray(np.asarray(b2, np.float32)),
        "b3": np.ascontiguousarray(np.asarray(b3, np.float32)),
    }
    in_maps = []
    for c in range(N_CORES):
        m = dict(shared)
        m["x"] = x[B_LOC * c:B_LOC * (c + 1), :]
        in_maps.append(m)
    return in_maps


def kernel(x, W0, b0, W1, b1, W2, b2, W3, b3):
    global _NC
    if _NC is None:
        _NC = _build_program()
    in_maps = build_in_maps(x, W0, b0, W1, b1, W2, b2, W3, b3)
    res = run_bass_kernel_spmd(_NC, in_maps, list(range(N_CORES)))
    return np.concatenate([res.results[c]["y"] for c in range(N_CORES)],
                          axis=0)
